# revision 12
# baseline (speedup 1.0000x reference)
"""Trainium2 Bass kernel for nn_CandidateFilterModel (segment_reduce).

Strategy (8 cores, S-column sharding for the heavy phases, pair sharding for the tail):
  - Core k owns sequence-column slice s_k = [256k, 256k+256).
  - Phase 1: entity aggregation.
      ent_att (local s-slice) = OH_mean-matmul of host-pregathered mention
      attention rows (fp8), streamed tile-by-tile, entity-half (et) outer so
      PSUM fits and the first matmul fires ~2us in.
      ent_emb^T = Ln of (exp(seq[mention_idx]) x OH_sum) matmuls emitted
      directly in [h-part, E] layout (64 N=256 matmuls); they fill the PE
      idle time under the even pair-gather window.
  - Phase 2: pair products. For all 2048 pairs: gather ent_att rows of head/
      tail entity (4KB fp8 rows cast to bf16, indirect DMA), multiply on DVE
      (bf16 2x mode), one DVE add folds 16 heads -> 8, then PE transpose-
      ACCUMULATE matmuls (x identity) fold the remaining 8 head-blocks while
      transposing -> raw^T in PSUM.
      EW = ent_emb @ W_head/W_tail is emitted between the even and odd tile
      groups so it executes under the odd gather window.
  - Phase 3: TWO AllToAlls (even pair-tiles = first 128 pairs of each dest
      core, then odd) redistribute raw^T so core k holds raw^T[:, P_k].
  - Phases 4-6 (per pair-half): contexts via seq^T-matmul, normalize, z_s/z_o
      via EW-gather one-hot matmuls + W_ctx matmuls + tanh, bilinear via
      W_bil matmuls + elementwise + ones-reduction matmul.
Host pre-casts: attention fp8 e4m3 (quantization error largely cancels in the
pair_att normalization), seq/weights bf16; mention rows (attention + seq) are
host-pregathered (pure indexing, like the one-hot/offset tensors).
DMA queues: gpsimd = pair gathers + collectives; sync = phase-1 loads,
staging, deferred tail loads (seqx/w_ctx/w_bil ride behind the even-tile
stagings so they don't steal HBM from the gathers); scalar = seqm + EW
weights; vector = paT reads (so a2a completion doesn't block other queues).
PSUM->SBUF copies ride the scalar engine to keep DVE free.
"""
import sys
import types
import numpy as np

S, H, HEADS = 2048, 1024, 16
E, NM, P = 256, 1024, 2048
PH = 1024
NC = 8
SL = S // NC          # 256 s-columns per core
PL = P // NC          # 256 pairs per core
NMT = NM // 128       # 8 mention tiles
NPT = P // 128        # 16 pair tiles
HS = HEADS * SL       # 4096 = width of per-core ent_att rows

_CACHE = {}


def _ensure_axon_profile_hook():
    """bass_utils' trace path imports antenv.axon_hooks, absent in this image."""
    if 'antenv.axon_hooks' in sys.modules:
        return
    try:
        import antenv.axon_hooks  # noqa: F401
        return
    except ImportError:
        pass
    mod = types.ModuleType('antenv.axon_hooks')
    holder = [None]
    mod.set_axon_ntff_profile_hook = lambda h: holder.__setitem__(0, h)
    mod.get_axon_ntff_profile_hook = lambda: holder[0]
    sys.modules['antenv.axon_hooks'] = mod
    try:
        from trn_agent_boot.trn_boot import _ntff_profile_via_ctypes
        hook = _ntff_profile_via_ctypes('/opt/axon/libaxon_pjrt.so')
        if hook is not None:
            mod.set_axon_ntff_profile_hook(hook)
    except Exception:
        pass


def _build(mt_ets, debug=False):
    """mt_ets: per mention-tile, tuple of entity-128-halves it touches."""
    import concourse.bass as bass
    import concourse.bacc as bacc
    import concourse.tile as tile
    from concourse import mybir
    from concourse.masks import make_identity

    F32 = mybir.dt.float32
    BF16 = mybir.dt.bfloat16
    F8 = mybir.dt.float8e4
    I32 = mybir.dt.int32
    AF = mybir.ActivationFunctionType
    OP = mybir.AluOpType

    nc = bacc.Bacc(num_devices=NC)

    # ---------------- inputs ----------------
    ag_k = nc.declare_dram_parameter("ag_k", [128, NMT * HS], F8, isOutput=False)
    seqm = nc.declare_dram_parameter("seqm", [128, NMT * H], BF16, isOutput=False)
    seqp = nc.declare_dram_parameter("seqp", [128, (S // 128) * H], BF16, isOutput=False)
    p_off = nc.declare_dram_parameter("p_off", [128, 2 * NPT], I32, isOutput=False)
    ohs = nc.declare_dram_parameter("ohs", [128, NMT * E], F8, isOutput=False)
    ohm = nc.declare_dram_parameter("ohm", [128, NMT * E], F8, isOutput=False)
    has0r = nc.declare_dram_parameter("has0r", [1, E], F32, isOutput=False)
    ohh_k = nc.declare_dram_parameter("ohh_k", [128, 2 * PL], BF16, isOutput=False)
    oht_k = nc.declare_dram_parameter("oht_k", [128, 2 * PL], BF16, isOutput=False)
    w_head = nc.declare_dram_parameter("w_head", [128, (H // 128) * PH], BF16, isOutput=False)
    w_tail = nc.declare_dram_parameter("w_tail", [128, (H // 128) * PH], BF16, isOutput=False)
    w_ctx = nc.declare_dram_parameter("w_ctx", [128, (H // 128) * PH], BF16, isOutput=False)
    w_bil = nc.declare_dram_parameter("w_bil", [128, (PH // 128) * PH], BF16, isOutput=False)
    b_head = nc.declare_dram_parameter("b_head", [128, PH // 128], F32, isOutput=False)
    b_tail = nc.declare_dram_parameter("b_tail", [128, PH // 128], F32, isOutput=False)
    b_bil = nc.declare_dram_parameter("b_bil", [1, 1], F32, isOutput=False)
    out = nc.declare_dram_parameter("out", [1, PL], F32, isOutput=True)

    dbg = {}
    if debug:
        dbg["ent_embT"] = nc.declare_dram_parameter("d_ent_embT", [H, E], BF16, isOutput=True)
        dbg["entA"] = nc.declare_dram_parameter("d_entA", [E, HS], BF16, isOutput=True)
        dbg["rawT"] = nc.declare_dram_parameter("d_rawT", [128, 2 * NPT * 128], BF16, isOutput=True)
        dbg["ctxnT"] = nc.declare_dram_parameter("d_ctxnT", [H, PL], BF16, isOutput=True)
        dbg["zrec"] = nc.declare_dram_parameter("d_zrec", [128, 2], F32, isOutput=True)
        dbg["zsT"] = nc.declare_dram_parameter("d_zsT", [PH, PL], BF16, isOutput=True)

    # internal DRAM
    entA_dram = nc.dram_tensor("entA_dram", [E, HS], F8)
    a2a_in = [nc.dram_tensor(f"a2a{h}_in", [NC, SL, 128], BF16) for h in range(2)]
    a2a_out = [nc.dram_tensor(f"a2a{h}_out", [NC, SL, 128], BF16) for h in range(2)]
    ccw_in = nc.dram_tensor("ccw_in", [NC, 1, 128], BF16)
    ccw_out = nc.dram_tensor("ccw_out", [NC, 1, 128], BF16)

    et_mts = {0: [mt for mt in range(NMT) if 0 in mt_ets[mt]],
              1: [mt for mt in range(NMT) if 1 in mt_ets[mt]]}

    with tile.TileContext(nc) as tc:
        with tc.tile_pool(name="singles", bufs=1) as singles, \
             tc.tile_pool(name="wpool", bufs=1) as wpool:
            # ---------------- phase 0: small loads (sync queue) ----------------
            p_off_t = singles.tile([128, 2 * NPT], I32)
            nc.sync.dma_start(out=p_off_t, in_=p_off[:, :])
            ohh_t = singles.tile([128, 2, PL], BF16)
            nc.sync.dma_start(out=ohh_t, in_=ohh_k[:, :])
            oht_t = singles.tile([128, 2, PL], BF16)
            nc.sync.dma_start(out=oht_t, in_=oht_k[:, :])
            bhs_t = singles.tile([128, PH // 128], F32)
            nc.sync.dma_start(out=bhs_t, in_=b_head[:, :])
            bts_t = singles.tile([128, PH // 128], F32)
            nc.sync.dma_start(out=bts_t, in_=b_tail[:, :])
            bbil_t = singles.tile([1, 1], F32)
            nc.sync.dma_start(out=bbil_t, in_=b_bil[:, :])
            ident = singles.tile([128, 128], BF16)
            make_identity(nc, ident[:, :])
            # warm activation tables; Exp last = first real user
            warm = singles.tile([1, 8], F32)
            nc.vector.memset(warm[:, :], 0.0)
            nc.scalar.activation(out=warm[:, :], in_=warm[:, :], func=AF.Tanh)
            nc.scalar.activation(out=warm[:, :], in_=warm[:, :], func=AF.Ln)
            nc.scalar.activation(out=warm[:, :], in_=warm[:, :], func=AF.Exp)
            ones_col = singles.tile([128, 1], BF16)
            nc.vector.memset(ones_col[:, :], 1.0)
            # CC warmup: a tiny AllToAll during phase 1 absorbs the ~11.5us
            # first-collective latency so a2a #A starts promptly.
            ccw_sb = singles.tile([1, NC * 128], BF16)
            nc.vector.memset(ccw_sb[:, :], 0.0)
            nc.sync.dma_start(out=ccw_in.rearrange("j o q -> o (j q)"), in_=ccw_sb)
            nc.gpsimd.collective_compute(
                "AllToAll", OP.bypass, replica_groups=[list(range(NC))],
                ins=[ccw_in[:, :, :]], outs=[ccw_out[:, :, :]])

            entTe = singles.tile([128, H // 128, E], BF16)  # ent_emb^T [hcol-part, hc, e]
            rawT = singles.tile([128, 2, NPT, 128], BF16)   # [s-part, sh, pt, p-row]
            paT = singles.tile([128, S // 128, PL], BF16)   # raw^T for my pairs, all s
            ctxT = singles.tile([128, H // 128, 128], BF16)
            ctxp_sb = singles.tile([128, H], BF16)          # normalized contexts [p, h]
            zsT = singles.tile([128, PH // 128, 128], BF16)
            zoT = singles.tile([128, PH // 128, 128], BF16)
            EWh = singles.tile([128, 2, PH], BF16)          # ent_emb @ W_head [e-part, et, PH]
            EWt = singles.tile([128, 2, PH], BF16)
            lg_sb = singles.tile([1, PL], F32)
            dbg_zs = singles.tile([128, PH // 128, PL], BF16) if debug else None
            dbg_ctx = singles.tile([128, H // 128, PL], BF16) if debug else None
            dbg_zr = singles.tile([128, 2], F32) if debug else None

            whb = wpool.tile([128, H // 128, PH], BF16)
            wtb = wpool.tile([128, H // 128, PH], BF16)
            seqx = wpool.tile([128, S // 128, H], BF16)
            wcb = wpool.tile([128, H // 128, PH], BF16)
            wbb = wpool.tile([128, PH // 128, PH], BF16)

            # ---------------- phase 1: aggregation + lse ----------
            with tc.tile_pool(name="p1", bufs=1) as p1:
                # seqm first on the scalar HWDGE queue (exp needs it ~5us in),
                # then the weight/seq loads (they fit under the agg window).
                entA_sb = p1.tile([128, 2, HS], F8)
                ohm_t = p1.tile([128, NMT, E], F8)
                nc.sync.dma_start(out=ohm_t, in_=ohm[:, :])
                ohs_t = p1.tile([128, NMT, E], F8)
                nc.sync.dma_start(out=ohs_t, in_=ohs[:, :])
                has0b = p1.tile([128, E], F32)
                nc.sync.dma_start(out=has0b, in_=has0r[:, :].to_broadcast([128, E]))

                # entity attention aggregation: et outer (PSUM = 2 hg x 8KB),
                # all mention tiles resident, loaded in two chunks (et0's
                # prefix first) so the et0 chains start early and the matmuls
                # run back-to-back at full PE clock.
                ag_all = p1.tile([128, NMT, HS], F8)
                split = (max(et_mts[0]) + 1) if et_mts[0] else 0
                if split > 0:
                    nc.scalar.dma_start(out=ag_all[:, 0:split, :],
                                        in_=ag_k[:, 0:split * HS])
                if split < NMT:
                    nc.scalar.dma_start(out=ag_all[:, split:NMT, :],
                                        in_=ag_k[:, split * HS:NMT * HS])
                # remaining loads fill the otherwise-idle DMA window under agg
                seqm_t = p1.tile([128, NMT, H], BF16)
                nc.scalar.dma_start(out=seqm_t, in_=seqm[:, :])
                nc.scalar.dma_start(out=whb, in_=w_head[:, :])
                nc.scalar.dma_start(out=wtb, in_=w_tail[:, :])
                nc.scalar.dma_start(out=seqx, in_=seqp[:, :])
                nc.scalar.dma_start(out=wcb, in_=w_ctx[:, :])
                nc.scalar.dma_start(out=wbb, in_=w_bil[:, :])

                # exp(seq[mention_idx]) in place on scalar
                for mt in range(NMT):
                    nc.scalar.activation(out=seqm_t[:, mt, :], in_=seqm_t[:, mt, :],
                                         func=AF.Exp)

                with tc.tile_pool(name="ps_a", bufs=1, space="PSUM") as ps_a:
                    for et in range(2):
                        mts = et_mts[et]
                        pas0 = ps_a.tile([128, 8 * SL], F32, space="PSUM", tag="agg0")
                        pas1 = ps_a.tile([128, 8 * SL], F32, space="PSUM", tag="agg1")
                        pas = {0: pas0, 1: pas1}
                        if not mts:
                            for hg in range(2):
                                nc.vector.memset(pas[hg][:, :], 0.0)
                        for mi, mt in enumerate(mts):
                            for hg in range(2):
                                for nch in range(4):
                                    nc.tensor.matmul(
                                        pas[hg][:, nch * 512:(nch + 1) * 512],
                                        ohm_t[:, mt, et * 128:(et + 1) * 128],
                                        ag_all[:, mt, hg * 2048 + nch * 512:
                                               hg * 2048 + (nch + 1) * 512],
                                        start=(mi == 0), stop=(mi == len(mts) - 1))
                        for hg in range(2):
                            nc.scalar.copy(
                                out=entA_sb[:, et, hg * 2048:(hg + 1) * 2048],
                                in_=pas[hg][:, :])
                            nc.sync.dma_start(
                                out=entA_dram.rearrange("(t p) w -> p t w", p=128)[
                                    :, et, hg * 2048:(hg + 1) * 2048],
                                in_=entA_sb[:, et, hg * 2048:(hg + 1) * 2048])
                            if debug:
                                eAb = p1.tile([128, 8 * SL], BF16, tag="entA_dbg")
                                nc.vector.tensor_copy(out=eAb[:, :], in_=pas[hg][:, :])
                                nc.sync.dma_start(
                                    out=dbg["entA"].rearrange("(t p) w -> p t w", p=128)[
                                        :, et, hg * 2048:(hg + 1) * 2048],
                                    in_=eAb[:, :])

                # logsumexp sums, transposed layout: sums^T[h, e] =
                # sum_m exp(seq_m)[m, h] ohs[m, e]. Emitted after agg so these
                # matmuls fill PE idle time under the even pair gathers.
                with tc.tile_pool(name="ps_l", bufs=1, space="PSUM") as ps_l:
                    for hc in range(H // 128):
                        sps = ps_l.tile([128, E], F32, space="PSUM", tag=f"s{hc}")
                        for mt in range(NMT):
                            nc.tensor.matmul(
                                sps[:, :], seqm_t[:, mt, hc * 128:(hc + 1) * 128],
                                ohs_t[:, mt, :], start=(mt == 0),
                                stop=(mt == NMT - 1))
                        nc.vector.tensor_tensor(out=sps[:, :], in0=sps[:, :],
                                                in1=has0b[:, :], op=OP.add)
                        nc.scalar.activation(out=entTe[:, hc, :], in_=sps[:, :],
                                             func=AF.Ln)
                if debug:
                    nc.sync.dma_start(
                        out=dbg["ent_embT"].rearrange("(t p) e -> p t e", p=128), in_=entTe)

            # ---------------- phase 2: pair products ----------------
            # evens (tiles 0,2,..,14 = first 128 pairs of each dest core) first
            # so AllToAll #A can fire while the odds still stream.
            def pair_tile(pt, pg, prod, ps_r):
                th = pg.tile([128, HS], BF16, tag="th")
                nc.gpsimd.indirect_dma_start(
                    out=th[:, :], out_offset=None, in_=entA_dram[:, :],
                    in_offset=bass.IndirectOffsetOnAxis(
                        ap=p_off_t[:, 2 * pt:2 * pt + 1], axis=0))
                tt = pg.tile([128, HS], BF16, tag="tt")
                nc.gpsimd.indirect_dma_start(
                    out=tt[:, :], out_offset=None, in_=entA_dram[:, :],
                    in_offset=bass.IndirectOffsetOnAxis(
                        ap=p_off_t[:, 2 * pt + 1:2 * pt + 2], axis=0))
                pr = prod.tile([128, HS], BF16, tag="pr")
                nc.vector.tensor_tensor(out=pr[:, :], in0=th[:, :], in1=tt[:, :],
                                        op=OP.mult)
                # fold 16 heads -> 8 on DVE; remaining 8 fold inside the
                # transpose-accumulate matmuls (x identity) on PE.
                nc.vector.tensor_tensor(out=pr[:, :8 * SL], in0=pr[:, :8 * SL],
                                        in1=pr[:, 8 * SL:], op=OP.add)
                rp = ps_r.tile([128, 2, 128], F32, space="PSUM", tag="rp")
                for sh in range(2):
                    for hb in range(8):
                        nc.tensor.matmul(
                            rp[:, sh, :],
                            pr[:, hb * SL + sh * 128: hb * SL + sh * 128 + 128],
                            ident[:, :], start=(hb == 0), stop=(hb == 7))
                    nc.scalar.copy(out=rawT[:, sh, pt, :], in_=rp[:, sh, :])
                c, odd = pt // 2, pt % 2
                nc.sync.dma_start(
                    out=a2a_in[odd][c].rearrange("(sh sp) p -> sp sh p", sh=2),
                    in_=rawT[:, :, pt, :])

            def ew_chain(ps_e, wsb, dstw, et):
                # one (W, et) chain of EW = ent_emb @ W: ~6us of PE, spread
                # between pair tiles so no single block dams the pipeline.
                # Shares the psA "ucp" banks (sequential uses, copy-drained).
                ep = ps_e.tile([128, PH], F32, space="PSUM", tag="ucp")
                for kt in range(H // 128):
                    for nch in range(2):
                        nc.tensor.matmul(
                            ep[:, nch * 512:(nch + 1) * 512],
                            entTe[:, kt, et * 128:(et + 1) * 128],
                            wsb[:, kt, nch * 512:(nch + 1) * 512],
                            start=(kt == 0), stop=(kt == H // 128 - 1))
                nc.scalar.copy(out=dstw[:, et, :], in_=ep[:, :])

            with tc.tile_pool(name="pg", bufs=3) as pg, \
                 tc.tile_pool(name="prod", bufs=2) as prod, \
                 tc.tile_pool(name="ps_r", bufs=2, space="PSUM") as ps_r, \
                 tc.tile_pool(name="psA", bufs=1, space="PSUM") as psA, \
                 tc.tile_pool(name="psB", bufs=2, space="PSUM") as psB, \
                 tc.tile_pool(name="zscr", bufs=2) as zscr:

                # ---------------- tail pieces (per pair-half) ----------------
                def tail_z(hf):
                    q0, q1 = hf * 128, hf * 128 + 128
                    zp2 = psA.tile([128, 1], F32, space="PSUM", tag="z2")
                    for t in range(S // 128):
                        nc.tensor.matmul(
                            zp2[:, :], paT[:, t, q0:q1], ones_col[:, :],
                            start=(t == 0), stop=(t == S // 128 - 1))
                    zr = zscr.tile([128, 1], F32, tag="zr")
                    nc.vector.tensor_scalar_add(out=zr[:, :], in0=zp2[:, :],
                                                scalar1=1e-6)
                    nc.vector.reciprocal(out=zr[:, :], in_=zr[:, :])
                    if debug:
                        nc.vector.tensor_copy(out=dbg_zr[:, hf:hf + 1], in_=zr[:, :])
                    return zr

                def tail_ucp_ctx(hf, zr):
                    q0, q1 = hf * 128, hf * 128 + 128
                    ucp = psA.tile([128, H], F32, space="PSUM", tag="ucp")
                    for t in range(S // 128):
                        for nchu in range(2):
                            nc.tensor.matmul(
                                ucp[:, nchu * 512:(nchu + 1) * 512],
                                paT[:, t, q0:q1],
                                seqx[:, t, nchu * 512:(nchu + 1) * 512],
                                start=(t == 0), stop=(t == S // 128 - 1))
                    # normalize on scalar (per-partition scale), transpose back
                    nc.scalar.activation(out=ctxp_sb[:, :], in_=ucp[:, :],
                                         func=AF.Copy, scale=zr[:, :])
                    for mc in range(H // 128):
                        tw = psB.tile([128, 128], F32, space="PSUM", tag="work")
                        nc.tensor.matmul(tw[:, :], ctxp_sb[:, mc * 128:(mc + 1) * 128],
                                         ident[:, :], start=True, stop=True)
                        nc.scalar.copy(out=ctxT[:, mc, :], in_=tw[:, :])
                        if debug:
                            nc.vector.tensor_copy(out=dbg_ctx[:, mc, q0:q1],
                                                  in_=ctxT[:, mc, :])

                def tail_zs(hf, jts):
                    q0, q1 = hf * 128, hf * 128 + 128
                    for jt in jts:
                        cps = psB.tile([128, 128], F32, space="PSUM", tag="work")
                        for kt in range(H // 128):
                            nc.tensor.matmul(
                                cps[:, :], wcb[:, kt, jt * 128:(jt + 1) * 128],
                                ctxT[:, kt, :], start=(kt == 0),
                                stop=(kt == H // 128 - 1))
                        cpsb = zscr.tile([128, 128], BF16, tag="cpsb")
                        nc.scalar.copy(out=cpsb[:, :], in_=cps[:, :])
                        for (ew, oh, bias, dstz) in ((EWh, ohh_t, bhs_t, zsT),
                                                     (EWt, oht_t, bts_t, zoT)):
                            zps = psB.tile([128, 128], F32, space="PSUM", tag="work")
                            for et in range(2):
                                nc.tensor.matmul(
                                    zps[:, :], ew[:, et, jt * 128:(jt + 1) * 128],
                                    oh[:, et, q0:q1], start=(et == 0), stop=(et == 1))
                            nc.vector.tensor_tensor(out=zps[:, :], in0=zps[:, :],
                                                    in1=cpsb[:, :], op=OP.add)
                            nc.scalar.activation(out=dstz[:, jt, :], in_=zps[:, :],
                                                 func=AF.Tanh, bias=bias[:, jt:jt + 1])
                        if debug:
                            nc.vector.tensor_copy(out=dbg_zs[:, jt, q0:q1],
                                                  in_=zsT[:, jt, :])

                def tail_bil(hf):
                    q0, q1 = hf * 128, hf * 128 + 128
                    lg = psA.tile([1, 128], F32, space="PSUM", tag="lg")
                    for jt in range(PH // 128):
                        ups = psB.tile([128, 128], F32, space="PSUM", tag="work")
                        for it in range(PH // 128):
                            nc.tensor.matmul(
                                ups[:, :], wbb[:, it, jt * 128:(jt + 1) * 128],
                                zsT[:, it, :], start=(it == 0),
                                stop=(it == PH // 128 - 1))
                        pb = zscr.tile([128, 128], BF16, tag="pb")
                        nc.vector.tensor_tensor(out=pb[:, :], in0=ups[:, :],
                                                in1=zoT[:, jt, :], op=OP.mult)
                        nc.tensor.matmul(
                            lg[:, :], ones_col[:, :], pb[:, :],
                            start=(jt == 0), stop=(jt == PH // 128 - 1))
                    nc.vector.tensor_scalar_add(out=lg_sb[:, q0:q1], in0=lg[:, :],
                                                scalar1=bbil_t[:, 0:1])

                # ---------------- even pair tiles + EW chains ----------------
                for c in range(NC):
                    pair_tile(2 * c, pg, prod, ps_r)
                    if c == 2:
                        ew_chain(psA, whb, EWh, 0)
                    elif c == 4:
                        ew_chain(psA, whb, EWh, 1)
                    elif c == 6:
                        ew_chain(psA, wtb, EWt, 0)
                # a2a #A fires as soon as the evens are staged.
                nc.gpsimd.collective_compute(
                    "AllToAll", OP.bypass, replica_groups=[list(range(NC))],
                    ins=[a2a_in[0][:, :, :]], outs=[a2a_out[0][:, :, :]])
                # paT read ON THE GPSIMD QUEUE: intentionally pauses the odd
                # gather descgen until a2a #A completes, so the collective
                # transfer runs uncontended (~10us) instead of fighting the
                # gathers for DMA engines (~35us + full Q0 blockage).
                nc.gpsimd.dma_start(
                    out=paT[:, :, 0:128],
                    in_=a2a_out[0].rearrange("j (sh sp) q -> sp (j sh) q", sh=2))

                # ---------------- odd pair tiles + EWt + tail half 0 ---------
                zr0 = None
                for c in range(NC):
                    pair_tile(2 * c + 1, pg, prod, ps_r)
                    if c == 0:
                        ew_chain(psA, wtb, EWt, 1)
                    elif c == 1:
                        zr0 = tail_z(0)
                    elif c == 2:
                        tail_ucp_ctx(0, zr0)
                    elif c == 3:
                        tail_zs(0, range(0, 4))
                    elif c == 4:
                        tail_zs(0, range(4, 8))
                    elif c == 5:
                        tail_bil(0)
                nc.gpsimd.collective_compute(
                    "AllToAll", OP.bypass, replica_groups=[list(range(NC))],
                    ins=[a2a_in[1][:, :, :]], outs=[a2a_out[1][:, :, :]])
                nc.scalar.dma_start(
                    out=paT[:, :, 128:256],
                    in_=a2a_out[1].rearrange("j (sh sp) q -> sp (j sh) q", sh=2))
                if debug:
                    nc.sync.dma_start(
                        out=dbg["rawT"][:, :],
                        in_=rawT.rearrange("p a b c -> p (a b c)"))

                # ---------------- tail half 1 ----------------
                zr1 = tail_z(1)
                tail_ucp_ctx(1, zr1)
                tail_zs(1, range(PH // 128))
                tail_bil(1)
                if debug:
                    nc.sync.dma_start(
                        out=dbg["ctxnT"].rearrange("(t p) q -> p t q", p=128),
                        in_=dbg_ctx)
                    nc.sync.dma_start(out=dbg["zrec"][:, :], in_=dbg_zr)
                    nc.sync.dma_start(
                        out=dbg["zsT"].rearrange("(t p) q -> p t q", p=128), in_=dbg_zs)
                nc.sync.dma_start(out=out[:, :], in_=lg_sb)

    nc.finalize()
    return nc


def _get_nc(mt_ets, debug=False):
    key = ("nc", mt_ets, debug)
    if key not in _CACHE:
        _CACHE[key] = _build(mt_ets, debug)
    return _CACHE[key]


def _prep_in_maps(inputs):
    import ml_dtypes
    bf16 = ml_dtypes.bfloat16
    f8 = ml_dtypes.float8_e4m3

    att = np.asarray(inputs["attention"], np.float32)          # [16, 2048, 2048]
    seq = np.asarray(inputs["sequence_output"], np.float32)
    mention_idx = np.asarray(inputs["mention_idx"], np.int32)  # [1024]
    entity_ids = np.asarray(inputs["entity_ids"], np.int32)    # [1024]
    pair_h = np.asarray(inputs["pair_h"], np.int32)            # [2048]
    pair_t = np.asarray(inputs["pair_t"], np.int32)

    def pm(x, t):
        """[(t*128), f...] -> partition-major [128, t*f] contiguous rows."""
        f = x.size // (t * 128)
        return np.ascontiguousarray(
            x.reshape(t, 128, f).transpose(1, 0, 2)).reshape(128, t * f)

    counts = np.bincount(entity_ids, minlength=E).astype(np.float32)
    inv_cnt = 1.0 / np.maximum(counts, 1.0)

    ohm = np.zeros((NM, E), np.float32)
    ohm[np.arange(NM), entity_ids] = inv_cnt[entity_ids]
    ohs_np = np.zeros((NM, E), np.float32)
    ohs_np[np.arange(NM), entity_ids] = 1.0
    has0r = (counts == 0).astype(np.float32)[None, :]

    # which entity-128-halves each mention tile touches (all-zero slabs skipped)
    mt_ets = tuple(
        tuple(sorted(set((entity_ids[mt * 128:(mt + 1) * 128] // 128).tolist())))
        for mt in range(NMT))

    order = np.argsort(pair_h, kind="stable")
    sph = pair_h[order]
    spt = pair_t[order]
    p_off = np.zeros((128, 2 * NPT), np.int32)
    for pt in range(NPT):
        seg = slice(pt * 128, (pt + 1) * 128)
        p_off[:, 2 * pt] = sph[seg]
        p_off[:, 2 * pt + 1] = spt[seg]

    # host-pregathered mention rows (pure indexing + dtype cast)
    att8_m = att[:, mention_idx, :].astype(f8)                 # [16, NM, 2048]
    seq_m = seq[mention_idx].astype(bf16)                      # [NM, H]

    shared = {
        "seqm": pm(seq_m, NMT),
        "seqp": pm(seq.astype(bf16), S // 128),
        "p_off": p_off,
        "ohm": pm(ohm.astype(f8), NMT),
        "ohs": pm(ohs_np.astype(f8), NMT),
        "has0r": has0r,
        "w_head": pm(np.asarray(inputs["W_head"], np.float32).astype(bf16), H // 128),
        "w_tail": pm(np.asarray(inputs["W_tail"], np.float32).astype(bf16), H // 128),
        "w_ctx": pm(np.asarray(inputs["W_ctx"], np.float32).astype(bf16), H // 128),
        "w_bil": pm(np.asarray(inputs["W_bil"], np.float32).astype(bf16), PH // 128),
        "b_head": np.asarray(inputs["b_head"], np.float32).reshape(PH // 128, 128).T.copy(),
        "b_tail": np.asarray(inputs["b_tail"], np.float32).reshape(PH // 128, 128).T.copy(),
        "b_bil": np.asarray(inputs["b_bil"], np.float32).reshape(1, 1),
    }

    in_maps = []
    for k in range(NC):
        sk = k * SL
        ag_kk = np.ascontiguousarray(
            att8_m[:, :, sk:sk + SL].transpose(1, 0, 2)).reshape(NM, HS)
        ohh_kk = np.zeros((E, PL), np.float32)
        ohh_kk[sph[k * PL:(k + 1) * PL], np.arange(PL)] = 1.0
        oht_kk = np.zeros((E, PL), np.float32)
        oht_kk[spt[k * PL:(k + 1) * PL], np.arange(PL)] = 1.0
        m = dict(shared)
        m["ag_k"] = pm(ag_kk, NMT)
        m["ohh_k"] = pm(ohh_kk.astype(bf16), 2)
        m["oht_k"] = pm(oht_kk.astype(bf16), 2)
        in_maps.append(m)
    return in_maps, mt_ets


def _run(inputs, trace=False, debug=False):
    _ensure_axon_profile_hook()
    from concourse.bass_utils import run_bass_kernel_spmd
    in_maps, mt_ets = _prep_in_maps(inputs)
    nc = _get_nc(mt_ets, debug)
    res = run_bass_kernel_spmd(nc, in_maps, list(range(NC)), trace=trace)
    sorted_logits = np.concatenate([np.asarray(res.results[k]["out"][0], np.float32)
                                    for k in range(NC)])
    order = np.argsort(np.asarray(inputs["pair_h"], np.int32), kind="stable")
    logits = np.empty(P, np.float32)
    logits[order] = sorted_logits
    return logits, res


def kernel(**inputs) -> np.ndarray:
    logits, _ = _run(inputs, trace=False)
    return logits


def kernel_traced(**inputs):
    logits, res = _run(inputs, trace=True)
    return logits, res


def kernel_debug(**inputs):
    logits, res = _run(inputs, trace=False, debug=True)
    return logits, res


# revision 13
# speedup vs baseline: 1.6195x; 1.6195x over previous
"""Trainium2 Bass kernel for nn_CandidateFilterModel (segment_reduce).

Strategy (8 cores, S-column sharding for the heavy phases, pair sharding for the tail):
  - Core k owns sequence-column slice s_k = [256k, 256k+256).
  - Phase 1: entity aggregation.
      ent_att (local s-slice) = OH_mean-matmul of host-pregathered mention
      attention rows (fp8), streamed tile-by-tile, entity-half (et) outer so
      PSUM fits and the first matmul fires ~2us in.
      ent_emb^T = Ln of (exp(seq[mention_idx]) x OH_sum) matmuls emitted
      directly in [h-part, E] layout (64 N=256 matmuls); they fill the PE
      idle time under the even pair-gather window.
  - Phase 2: pair products. For all 2048 pairs: gather ent_att rows of head/
      tail entity (4KB fp8 rows cast to bf16, indirect DMA), multiply on DVE
      (bf16 2x mode), one DVE add folds 16 heads -> 8, then PE transpose-
      ACCUMULATE matmuls (x identity) fold the remaining 8 head-blocks while
      transposing -> raw^T in PSUM.
      EW = ent_emb @ W_head/W_tail is emitted between the even and odd tile
      groups so it executes under the odd gather window.
  - Phase 3: TWO AllToAlls (even pair-tiles = first 128 pairs of each dest
      core, then odd) redistribute raw^T so core k holds raw^T[:, P_k].
  - Phases 4-6 (per pair-half): contexts via seq^T-matmul, normalize, z_s/z_o
      via EW-gather one-hot matmuls + W_ctx matmuls + tanh, bilinear via
      W_bil matmuls + elementwise + ones-reduction matmul.
Host pre-casts: attention fp8 e4m3 (quantization error largely cancels in the
pair_att normalization), seq/weights bf16; mention rows (attention + seq) are
host-pregathered (pure indexing, like the one-hot/offset tensors).
DMA queues: gpsimd = pair gathers + collectives; sync = phase-1 loads,
staging, deferred tail loads (seqx/w_ctx/w_bil ride behind the even-tile
stagings so they don't steal HBM from the gathers); scalar = seqm + EW
weights; vector = paT reads (so a2a completion doesn't block other queues).
PSUM->SBUF copies ride the scalar engine to keep DVE free.
"""
import sys
import types
import numpy as np

S, H, HEADS = 2048, 1024, 16
E, NM, P = 256, 1024, 2048
PH = 1024
NC = 8
SL = S // NC          # 256 s-columns per core
PL = P // NC          # 256 pairs per core
NMT = NM // 128       # 8 mention tiles
NPT = P // 128        # 16 pair tiles
HS = HEADS * SL       # 4096 = width of per-core ent_att rows

_CACHE = {}


def _ensure_axon_profile_hook():
    """bass_utils' trace path imports antenv.axon_hooks, absent in this image."""
    if 'antenv.axon_hooks' in sys.modules:
        return
    try:
        import antenv.axon_hooks  # noqa: F401
        return
    except ImportError:
        pass
    mod = types.ModuleType('antenv.axon_hooks')
    holder = [None]
    mod.set_axon_ntff_profile_hook = lambda h: holder.__setitem__(0, h)
    mod.get_axon_ntff_profile_hook = lambda: holder[0]
    sys.modules['antenv.axon_hooks'] = mod
    try:
        from trn_agent_boot.trn_boot import _ntff_profile_via_ctypes
        hook = _ntff_profile_via_ctypes('/opt/axon/libaxon_pjrt.so')
        if hook is not None:
            mod.set_axon_ntff_profile_hook(hook)
    except Exception:
        pass


def _build(mt_ets, debug=False):
    """mt_ets: per mention-tile, tuple of entity-128-halves it touches."""
    import concourse.bass as bass
    import concourse.bacc as bacc
    import concourse.tile as tile
    from concourse import mybir
    from concourse.masks import make_identity

    F32 = mybir.dt.float32
    BF16 = mybir.dt.bfloat16
    F8 = mybir.dt.float8e4
    I32 = mybir.dt.int32
    AF = mybir.ActivationFunctionType
    OP = mybir.AluOpType

    nc = bacc.Bacc(num_devices=NC)

    # ---------------- inputs ----------------
    ag_k = nc.declare_dram_parameter("ag_k", [128, NMT * HS], F8, isOutput=False)
    seqm = nc.declare_dram_parameter("seqm", [128, NMT * H], BF16, isOutput=False)
    seqp = nc.declare_dram_parameter("seqp", [128, (S // 128) * H], BF16, isOutput=False)
    p_off = nc.declare_dram_parameter("p_off", [128, 2 * NPT], I32, isOutput=False)
    ohs = nc.declare_dram_parameter("ohs", [128, NMT * E], F8, isOutput=False)
    ohm = nc.declare_dram_parameter("ohm", [128, NMT * E], F8, isOutput=False)
    has0r = nc.declare_dram_parameter("has0r", [1, E], F32, isOutput=False)
    ohh_k = nc.declare_dram_parameter("ohh_k", [128, 2 * PL], BF16, isOutput=False)
    oht_k = nc.declare_dram_parameter("oht_k", [128, 2 * PL], BF16, isOutput=False)
    w_head = nc.declare_dram_parameter("w_head", [128, (H // 128) * PH], BF16, isOutput=False)
    w_tail = nc.declare_dram_parameter("w_tail", [128, (H // 128) * PH], BF16, isOutput=False)
    w_ctx = nc.declare_dram_parameter("w_ctx", [128, (H // 128) * PH], BF16, isOutput=False)
    w_bil = nc.declare_dram_parameter("w_bil", [128, (PH // 128) * PH], BF16, isOutput=False)
    b_head = nc.declare_dram_parameter("b_head", [128, PH // 128], F32, isOutput=False)
    b_tail = nc.declare_dram_parameter("b_tail", [128, PH // 128], F32, isOutput=False)
    b_bil = nc.declare_dram_parameter("b_bil", [1, 1], F32, isOutput=False)
    out = nc.declare_dram_parameter("out", [1, PL], F32, isOutput=True)

    dbg = {}
    if debug:
        dbg["ent_embT"] = nc.declare_dram_parameter("d_ent_embT", [H, E], BF16, isOutput=True)
        dbg["entA"] = nc.declare_dram_parameter("d_entA", [E, HS], BF16, isOutput=True)
        dbg["rawT"] = nc.declare_dram_parameter("d_rawT", [128, 2 * NPT * 128], BF16, isOutput=True)
        dbg["ctxnT"] = nc.declare_dram_parameter("d_ctxnT", [H, PL], BF16, isOutput=True)
        dbg["zrec"] = nc.declare_dram_parameter("d_zrec", [128, 2], F32, isOutput=True)
        dbg["zsT"] = nc.declare_dram_parameter("d_zsT", [PH, PL], BF16, isOutput=True)

    # internal DRAM
    entA_dram = nc.dram_tensor("entA_dram", [E, HS], F8)
    a2a_in = [nc.dram_tensor(f"a2a{h}_in", [NC, SL, 128], BF16) for h in range(2)]
    a2a_out = [nc.dram_tensor(f"a2a{h}_out", [NC, SL, 128], BF16) for h in range(2)]

    et_mts = {0: [mt for mt in range(NMT) if 0 in mt_ets[mt]],
              1: [mt for mt in range(NMT) if 1 in mt_ets[mt]]}

    with tile.TileContext(nc) as tc:
        with tc.tile_pool(name="singles", bufs=1) as singles, \
             tc.tile_pool(name="wpool", bufs=1) as wpool:
            # ---------------- phase 0: small loads (sync queue) ----------------
            p_off_t = singles.tile([128, 2 * NPT], I32)
            nc.sync.dma_start(out=p_off_t, in_=p_off[:, :])
            ohh_t = singles.tile([128, 2, PL], BF16)
            nc.sync.dma_start(out=ohh_t, in_=ohh_k[:, :])
            oht_t = singles.tile([128, 2, PL], BF16)
            nc.sync.dma_start(out=oht_t, in_=oht_k[:, :])
            bhs_t = singles.tile([128, PH // 128], F32)
            nc.sync.dma_start(out=bhs_t, in_=b_head[:, :])
            bts_t = singles.tile([128, PH // 128], F32)
            nc.sync.dma_start(out=bts_t, in_=b_tail[:, :])
            bbil_t = singles.tile([1, 1], F32)
            nc.sync.dma_start(out=bbil_t, in_=b_bil[:, :])
            ident = singles.tile([128, 128], BF16)
            make_identity(nc, ident[:, :])
            # warm activation tables; Exp last = first real user
            warm = singles.tile([1, 8], F32)
            nc.vector.memset(warm[:, :], 0.0)
            nc.scalar.activation(out=warm[:, :], in_=warm[:, :], func=AF.Tanh)
            nc.scalar.activation(out=warm[:, :], in_=warm[:, :], func=AF.Ln)
            nc.scalar.activation(out=warm[:, :], in_=warm[:, :], func=AF.Exp)
            ones_col = singles.tile([128, 1], BF16)
            nc.vector.memset(ones_col[:, :], 1.0)

            entTe = singles.tile([128, H // 128, E], BF16)  # ent_emb^T [hcol-part, hc, e]
            rawT = singles.tile([128, 2, NPT, 128], BF16)   # [s-part, sh, pt, p-row]
            paT = singles.tile([128, S // 128, PL], BF16)   # raw^T for my pairs, all s
            ctxT = singles.tile([128, H // 128, 128], BF16)
            ctxp_sb = singles.tile([128, H], BF16)          # normalized contexts [p, h]
            zsT = singles.tile([128, PH // 128, 128], BF16)
            zoT = singles.tile([128, PH // 128, 128], BF16)
            EWh = singles.tile([128, 2, PH], BF16)          # ent_emb @ W_head [e-part, et, PH]
            EWt = singles.tile([128, 2, PH], BF16)
            lg_sb = singles.tile([1, PL], F32)
            dbg_zs = singles.tile([128, PH // 128, PL], BF16) if debug else None
            dbg_ctx = singles.tile([128, H // 128, PL], BF16) if debug else None
            dbg_zr = singles.tile([128, 2], F32) if debug else None

            whb = wpool.tile([128, H // 128, PH], BF16)
            wtb = wpool.tile([128, H // 128, PH], BF16)
            seqx = wpool.tile([128, S // 128, H], BF16)
            wcb = wpool.tile([128, H // 128, PH], BF16)
            wbb = wpool.tile([128, PH // 128, PH], BF16)

            # ---------------- phase 1: aggregation + lse ----------
            with tc.tile_pool(name="p1", bufs=1) as p1:
                # seqm first on the scalar HWDGE queue (exp needs it ~5us in),
                # then the weight/seq loads (they fit under the agg window).
                entA_sb = p1.tile([128, 2, HS], F8)
                ohm_t = p1.tile([128, NMT, E], F8)
                nc.sync.dma_start(out=ohm_t, in_=ohm[:, :])
                ohs_t = p1.tile([128, NMT, E], F8)
                nc.sync.dma_start(out=ohs_t, in_=ohs[:, :])
                has0b = p1.tile([128, E], F32)
                nc.sync.dma_start(out=has0b, in_=has0r[:, :].to_broadcast([128, E]))

                # entity attention aggregation: et outer (PSUM = 2 hg x 8KB),
                # all mention tiles resident, loaded in two chunks (et0's
                # prefix first) so the et0 chains start early and the matmuls
                # run back-to-back at full PE clock.
                ag_all = p1.tile([128, NMT, HS], F8)
                split = (max(et_mts[0]) + 1) if et_mts[0] else 0
                if split > 0:
                    nc.scalar.dma_start(out=ag_all[:, 0:split, :],
                                        in_=ag_k[:, 0:split * HS])
                if split < NMT:
                    nc.scalar.dma_start(out=ag_all[:, split:NMT, :],
                                        in_=ag_k[:, split * HS:NMT * HS])
                # remaining loads fill the otherwise-idle DMA window under agg
                seqm_t = p1.tile([128, NMT, H], BF16)
                nc.scalar.dma_start(out=seqm_t, in_=seqm[:, :])
                nc.scalar.dma_start(out=whb, in_=w_head[:, :])
                nc.scalar.dma_start(out=wtb, in_=w_tail[:, :])
                nc.scalar.dma_start(out=seqx, in_=seqp[:, :])
                nc.scalar.dma_start(out=wcb, in_=w_ctx[:, :])
                nc.scalar.dma_start(out=wbb, in_=w_bil[:, :])

                # exp(seq[mention_idx]) in place on scalar
                for mt in range(NMT):
                    nc.scalar.activation(out=seqm_t[:, mt, :], in_=seqm_t[:, mt, :],
                                         func=AF.Exp)

                with tc.tile_pool(name="ps_a", bufs=1, space="PSUM") as ps_a:
                    for et in range(2):
                        mts = et_mts[et]
                        pas0 = ps_a.tile([128, 8 * SL], F32, space="PSUM", tag="agg0")
                        pas1 = ps_a.tile([128, 8 * SL], F32, space="PSUM", tag="agg1")
                        pas = {0: pas0, 1: pas1}
                        if not mts:
                            for hg in range(2):
                                nc.vector.memset(pas[hg][:, :], 0.0)
                        for mi, mt in enumerate(mts):
                            for hg in range(2):
                                for nch in range(4):
                                    nc.tensor.matmul(
                                        pas[hg][:, nch * 512:(nch + 1) * 512],
                                        ohm_t[:, mt, et * 128:(et + 1) * 128],
                                        ag_all[:, mt, hg * 2048 + nch * 512:
                                               hg * 2048 + (nch + 1) * 512],
                                        start=(mi == 0), stop=(mi == len(mts) - 1))
                        for hg in range(2):
                            nc.scalar.copy(
                                out=entA_sb[:, et, hg * 2048:(hg + 1) * 2048],
                                in_=pas[hg][:, :])
                            nc.sync.dma_start(
                                out=entA_dram.rearrange("(t p) w -> p t w", p=128)[
                                    :, et, hg * 2048:(hg + 1) * 2048],
                                in_=entA_sb[:, et, hg * 2048:(hg + 1) * 2048])
                            if debug:
                                eAb = p1.tile([128, 8 * SL], BF16, tag="entA_dbg")
                                nc.vector.tensor_copy(out=eAb[:, :], in_=pas[hg][:, :])
                                nc.sync.dma_start(
                                    out=dbg["entA"].rearrange("(t p) w -> p t w", p=128)[
                                        :, et, hg * 2048:(hg + 1) * 2048],
                                    in_=eAb[:, :])

                # logsumexp sums, transposed layout: sums^T[h, e] =
                # sum_m exp(seq_m)[m, h] ohs[m, e]. Emitted after agg so these
                # matmuls fill PE idle time under the even pair gathers.
                with tc.tile_pool(name="ps_l", bufs=1, space="PSUM") as ps_l:
                    for hc in range(H // 128):
                        sps = ps_l.tile([128, E], F32, space="PSUM", tag=f"s{hc}")
                        for mt in range(NMT):
                            nc.tensor.matmul(
                                sps[:, :], seqm_t[:, mt, hc * 128:(hc + 1) * 128],
                                ohs_t[:, mt, :], start=(mt == 0),
                                stop=(mt == NMT - 1))
                        nc.vector.tensor_tensor(out=sps[:, :], in0=sps[:, :],
                                                in1=has0b[:, :], op=OP.add)
                        nc.scalar.activation(out=entTe[:, hc, :], in_=sps[:, :],
                                             func=AF.Ln)
                if debug:
                    nc.sync.dma_start(
                        out=dbg["ent_embT"].rearrange("(t p) e -> p t e", p=128), in_=entTe)

            # ---------------- phase 2: pair products ----------------
            # evens (tiles 0,2,..,14 = first 128 pairs of each dest core) first
            # so AllToAll #A can fire while the odds still stream.
            def pair_tile(pt, pg, prod, ps_r):
                th = pg.tile([128, HS], BF16, tag="th")
                nc.gpsimd.indirect_dma_start(
                    out=th[:, :], out_offset=None, in_=entA_dram[:, :],
                    in_offset=bass.IndirectOffsetOnAxis(
                        ap=p_off_t[:, 2 * pt:2 * pt + 1], axis=0))
                tt = pg.tile([128, HS], BF16, tag="tt")
                nc.gpsimd.indirect_dma_start(
                    out=tt[:, :], out_offset=None, in_=entA_dram[:, :],
                    in_offset=bass.IndirectOffsetOnAxis(
                        ap=p_off_t[:, 2 * pt + 1:2 * pt + 2], axis=0))
                pr = prod.tile([128, HS], BF16, tag="pr")
                nc.vector.tensor_tensor(out=pr[:, :], in0=th[:, :], in1=tt[:, :],
                                        op=OP.mult)
                # fold 16 heads -> 8 on DVE; remaining 8 fold inside the
                # transpose-accumulate matmuls (x identity) on PE.
                nc.vector.tensor_tensor(out=pr[:, :8 * SL], in0=pr[:, :8 * SL],
                                        in1=pr[:, 8 * SL:], op=OP.add)
                rp = ps_r.tile([128, 2, 128], F32, space="PSUM", tag="rp")
                for sh in range(2):
                    for hb in range(8):
                        nc.tensor.matmul(
                            rp[:, sh, :],
                            pr[:, hb * SL + sh * 128: hb * SL + sh * 128 + 128],
                            ident[:, :], start=(hb == 0), stop=(hb == 7))
                    nc.scalar.copy(out=rawT[:, sh, pt, :], in_=rp[:, sh, :])
                c, odd = pt // 2, pt % 2
                nc.sync.dma_start(
                    out=a2a_in[odd][c].rearrange("(sh sp) p -> sp sh p", sh=2),
                    in_=rawT[:, :, pt, :])

            def ew_chain(ps_e, wsb, dstw, et):
                # one (W, et) chain of EW = ent_emb @ W: ~6us of PE, spread
                # between pair tiles so no single block dams the pipeline.
                # Shares the psA "ucp" banks (sequential uses, copy-drained).
                ep = ps_e.tile([128, PH], F32, space="PSUM", tag="ucp")
                for kt in range(H // 128):
                    for nch in range(2):
                        nc.tensor.matmul(
                            ep[:, nch * 512:(nch + 1) * 512],
                            entTe[:, kt, et * 128:(et + 1) * 128],
                            wsb[:, kt, nch * 512:(nch + 1) * 512],
                            start=(kt == 0), stop=(kt == H // 128 - 1))
                nc.scalar.copy(out=dstw[:, et, :], in_=ep[:, :])

            with tc.tile_pool(name="pg", bufs=3) as pg, \
                 tc.tile_pool(name="prod", bufs=2) as prod, \
                 tc.tile_pool(name="ps_r", bufs=2, space="PSUM") as ps_r, \
                 tc.tile_pool(name="psA", bufs=1, space="PSUM") as psA, \
                 tc.tile_pool(name="psB", bufs=2, space="PSUM") as psB, \
                 tc.tile_pool(name="zscr", bufs=2) as zscr:

                # ---------------- tail pieces (per pair-half) ----------------
                def tail_z(hf):
                    q0, q1 = hf * 128, hf * 128 + 128
                    zp2 = psA.tile([128, 1], F32, space="PSUM", tag="z2")
                    for t in range(S // 128):
                        nc.tensor.matmul(
                            zp2[:, :], paT[:, t, q0:q1], ones_col[:, :],
                            start=(t == 0), stop=(t == S // 128 - 1))
                    zr = zscr.tile([128, 1], F32, tag="zr")
                    nc.vector.tensor_scalar_add(out=zr[:, :], in0=zp2[:, :],
                                                scalar1=1e-6)
                    nc.vector.reciprocal(out=zr[:, :], in_=zr[:, :])
                    if debug:
                        nc.vector.tensor_copy(out=dbg_zr[:, hf:hf + 1], in_=zr[:, :])
                    return zr

                def tail_ucp_ctx(hf, zr):
                    q0, q1 = hf * 128, hf * 128 + 128
                    ucp = psA.tile([128, H], F32, space="PSUM", tag="ucp")
                    for t in range(S // 128):
                        for nchu in range(2):
                            nc.tensor.matmul(
                                ucp[:, nchu * 512:(nchu + 1) * 512],
                                paT[:, t, q0:q1],
                                seqx[:, t, nchu * 512:(nchu + 1) * 512],
                                start=(t == 0), stop=(t == S // 128 - 1))
                    # normalize on scalar (per-partition scale), transpose back
                    nc.scalar.activation(out=ctxp_sb[:, :], in_=ucp[:, :],
                                         func=AF.Copy, scale=zr[:, :])
                    for mc in range(H // 128):
                        tw = psB.tile([128, 128], F32, space="PSUM", tag="work")
                        nc.tensor.matmul(tw[:, :], ctxp_sb[:, mc * 128:(mc + 1) * 128],
                                         ident[:, :], start=True, stop=True)
                        nc.scalar.copy(out=ctxT[:, mc, :], in_=tw[:, :])
                        if debug:
                            nc.vector.tensor_copy(out=dbg_ctx[:, mc, q0:q1],
                                                  in_=ctxT[:, mc, :])

                def tail_zs(hf, jts):
                    q0, q1 = hf * 128, hf * 128 + 128
                    for jt in jts:
                        cps = psB.tile([128, 128], F32, space="PSUM", tag="work")
                        for kt in range(H // 128):
                            nc.tensor.matmul(
                                cps[:, :], wcb[:, kt, jt * 128:(jt + 1) * 128],
                                ctxT[:, kt, :], start=(kt == 0),
                                stop=(kt == H // 128 - 1))
                        cpsb = zscr.tile([128, 128], BF16, tag="cpsb")
                        nc.scalar.copy(out=cpsb[:, :], in_=cps[:, :])
                        for (ew, oh, bias, dstz) in ((EWh, ohh_t, bhs_t, zsT),
                                                     (EWt, oht_t, bts_t, zoT)):
                            zps = psB.tile([128, 128], F32, space="PSUM", tag="work")
                            for et in range(2):
                                nc.tensor.matmul(
                                    zps[:, :], ew[:, et, jt * 128:(jt + 1) * 128],
                                    oh[:, et, q0:q1], start=(et == 0), stop=(et == 1))
                            nc.vector.tensor_tensor(out=zps[:, :], in0=zps[:, :],
                                                    in1=cpsb[:, :], op=OP.add)
                            nc.scalar.activation(out=dstz[:, jt, :], in_=zps[:, :],
                                                 func=AF.Tanh, bias=bias[:, jt:jt + 1])
                        if debug:
                            nc.vector.tensor_copy(out=dbg_zs[:, jt, q0:q1],
                                                  in_=zsT[:, jt, :])

                def tail_bil(hf):
                    q0, q1 = hf * 128, hf * 128 + 128
                    lg = psA.tile([1, 128], F32, space="PSUM", tag="lg")
                    for jt in range(PH // 128):
                        ups = psB.tile([128, 128], F32, space="PSUM", tag="work")
                        for it in range(PH // 128):
                            nc.tensor.matmul(
                                ups[:, :], wbb[:, it, jt * 128:(jt + 1) * 128],
                                zsT[:, it, :], start=(it == 0),
                                stop=(it == PH // 128 - 1))
                        pb = zscr.tile([128, 128], BF16, tag="pb")
                        nc.vector.tensor_tensor(out=pb[:, :], in0=ups[:, :],
                                                in1=zoT[:, jt, :], op=OP.mult)
                        nc.tensor.matmul(
                            lg[:, :], ones_col[:, :], pb[:, :],
                            start=(jt == 0), stop=(jt == PH // 128 - 1))
                    nc.vector.tensor_scalar_add(out=lg_sb[:, q0:q1], in0=lg[:, :],
                                                scalar1=bbil_t[:, 0:1])

                # ---------------- even pair tiles + EW chains ----------------
                for c in range(NC):
                    pair_tile(2 * c, pg, prod, ps_r)
                    if c == 2:
                        ew_chain(psA, whb, EWh, 0)
                    elif c == 4:
                        ew_chain(psA, whb, EWh, 1)
                    elif c == 6:
                        ew_chain(psA, wtb, EWt, 0)
                # a2a #A fires as soon as the evens are staged.
                nc.gpsimd.collective_compute(
                    "AllToAll", OP.bypass, replica_groups=[list(range(NC))],
                    ins=[a2a_in[0][:, :, :]], outs=[a2a_out[0][:, :, :]])
                # paT read ON THE GPSIMD QUEUE: intentionally pauses the odd
                # gather descgen until a2a #A completes, so the collective
                # transfer runs uncontended (~10us) instead of fighting the
                # gathers for DMA engines (~35us + full Q0 blockage).
                nc.gpsimd.dma_start(
                    out=paT[:, :, 0:128],
                    in_=a2a_out[0].rearrange("j (sh sp) q -> sp (j sh) q", sh=2))

                # ---------------- odd pair tiles + EWt + tail half 0 ---------
                zr0 = None
                for c in range(NC):
                    pair_tile(2 * c + 1, pg, prod, ps_r)
                    if c == 0:
                        ew_chain(psA, wtb, EWt, 1)
                    elif c == 1:
                        zr0 = tail_z(0)
                    elif c == 2:
                        tail_ucp_ctx(0, zr0)
                    elif c == 3:
                        tail_zs(0, range(0, 4))
                    elif c == 4:
                        tail_zs(0, range(4, 8))
                    elif c == 5:
                        tail_bil(0)
                nc.gpsimd.collective_compute(
                    "AllToAll", OP.bypass, replica_groups=[list(range(NC))],
                    ins=[a2a_in[1][:, :, :]], outs=[a2a_out[1][:, :, :]])
                nc.scalar.dma_start(
                    out=paT[:, :, 128:256],
                    in_=a2a_out[1].rearrange("j (sh sp) q -> sp (j sh) q", sh=2))
                if debug:
                    nc.sync.dma_start(
                        out=dbg["rawT"][:, :],
                        in_=rawT.rearrange("p a b c -> p (a b c)"))

                # ---------------- tail half 1 ----------------
                zr1 = tail_z(1)
                tail_ucp_ctx(1, zr1)
                tail_zs(1, range(PH // 128))
                tail_bil(1)
                if debug:
                    nc.sync.dma_start(
                        out=dbg["ctxnT"].rearrange("(t p) q -> p t q", p=128),
                        in_=dbg_ctx)
                    nc.sync.dma_start(out=dbg["zrec"][:, :], in_=dbg_zr)
                    nc.sync.dma_start(
                        out=dbg["zsT"].rearrange("(t p) q -> p t q", p=128), in_=dbg_zs)
                nc.sync.dma_start(out=out[:, :], in_=lg_sb)

    nc.finalize()
    return nc


def _get_nc(mt_ets, debug=False):
    key = ("nc", mt_ets, debug)
    if key not in _CACHE:
        _CACHE[key] = _build(mt_ets, debug)
    return _CACHE[key]


def _prep_in_maps(inputs):
    import ml_dtypes
    bf16 = ml_dtypes.bfloat16
    f8 = ml_dtypes.float8_e4m3

    att = np.asarray(inputs["attention"], np.float32)          # [16, 2048, 2048]
    seq = np.asarray(inputs["sequence_output"], np.float32)
    mention_idx = np.asarray(inputs["mention_idx"], np.int32)  # [1024]
    entity_ids = np.asarray(inputs["entity_ids"], np.int32)    # [1024]
    pair_h = np.asarray(inputs["pair_h"], np.int32)            # [2048]
    pair_t = np.asarray(inputs["pair_t"], np.int32)

    def pm(x, t):
        """[(t*128), f...] -> partition-major [128, t*f] contiguous rows."""
        f = x.size // (t * 128)
        return np.ascontiguousarray(
            x.reshape(t, 128, f).transpose(1, 0, 2)).reshape(128, t * f)

    counts = np.bincount(entity_ids, minlength=E).astype(np.float32)
    inv_cnt = 1.0 / np.maximum(counts, 1.0)

    ohm = np.zeros((NM, E), np.float32)
    ohm[np.arange(NM), entity_ids] = inv_cnt[entity_ids]
    ohs_np = np.zeros((NM, E), np.float32)
    ohs_np[np.arange(NM), entity_ids] = 1.0
    has0r = (counts == 0).astype(np.float32)[None, :]

    # which entity-128-halves each mention tile touches (all-zero slabs skipped)
    mt_ets = tuple(
        tuple(sorted(set((entity_ids[mt * 128:(mt + 1) * 128] // 128).tolist())))
        for mt in range(NMT))

    order = np.argsort(pair_h, kind="stable")
    sph = pair_h[order]
    spt = pair_t[order]
    p_off = np.zeros((128, 2 * NPT), np.int32)
    for pt in range(NPT):
        seg = slice(pt * 128, (pt + 1) * 128)
        p_off[:, 2 * pt] = sph[seg]
        p_off[:, 2 * pt + 1] = spt[seg]

    # host-pregathered mention rows (pure indexing + dtype cast)
    att8_m = att[:, mention_idx, :].astype(f8)                 # [16, NM, 2048]
    seq_m = seq[mention_idx].astype(bf16)                      # [NM, H]

    shared = {
        "seqm": pm(seq_m, NMT),
        "seqp": pm(seq.astype(bf16), S // 128),
        "p_off": p_off,
        "ohm": pm(ohm.astype(f8), NMT),
        "ohs": pm(ohs_np.astype(f8), NMT),
        "has0r": has0r,
        "w_head": pm(np.asarray(inputs["W_head"], np.float32).astype(bf16), H // 128),
        "w_tail": pm(np.asarray(inputs["W_tail"], np.float32).astype(bf16), H // 128),
        "w_ctx": pm(np.asarray(inputs["W_ctx"], np.float32).astype(bf16), H // 128),
        "w_bil": pm(np.asarray(inputs["W_bil"], np.float32).astype(bf16), PH // 128),
        "b_head": np.asarray(inputs["b_head"], np.float32).reshape(PH // 128, 128).T.copy(),
        "b_tail": np.asarray(inputs["b_tail"], np.float32).reshape(PH // 128, 128).T.copy(),
        "b_bil": np.asarray(inputs["b_bil"], np.float32).reshape(1, 1),
    }

    in_maps = []
    for k in range(NC):
        sk = k * SL
        ag_kk = np.ascontiguousarray(
            att8_m[:, :, sk:sk + SL].transpose(1, 0, 2)).reshape(NM, HS)
        ohh_kk = np.zeros((E, PL), np.float32)
        ohh_kk[sph[k * PL:(k + 1) * PL], np.arange(PL)] = 1.0
        oht_kk = np.zeros((E, PL), np.float32)
        oht_kk[spt[k * PL:(k + 1) * PL], np.arange(PL)] = 1.0
        m = dict(shared)
        m["ag_k"] = pm(ag_kk, NMT)
        m["ohh_k"] = pm(ohh_kk.astype(bf16), 2)
        m["oht_k"] = pm(oht_kk.astype(bf16), 2)
        in_maps.append(m)
    return in_maps, mt_ets


def _run(inputs, trace=False, debug=False):
    _ensure_axon_profile_hook()
    from concourse.bass_utils import run_bass_kernel_spmd
    in_maps, mt_ets = _prep_in_maps(inputs)
    nc = _get_nc(mt_ets, debug)
    res = run_bass_kernel_spmd(nc, in_maps, list(range(NC)), trace=trace)
    sorted_logits = np.concatenate([np.asarray(res.results[k]["out"][0], np.float32)
                                    for k in range(NC)])
    order = np.argsort(np.asarray(inputs["pair_h"], np.int32), kind="stable")
    logits = np.empty(P, np.float32)
    logits[order] = sorted_logits
    return logits, res


def kernel(**inputs) -> np.ndarray:
    logits, _ = _run(inputs, trace=False)
    return logits


def kernel_traced(**inputs):
    logits, res = _run(inputs, trace=True)
    return logits, res


def kernel_debug(**inputs):
    logits, res = _run(inputs, trace=False, debug=True)
    return logits, res


# revision 14
# speedup vs baseline: 1.7448x; 1.0774x over previous
"""Trainium2 Bass kernel for nn_CandidateFilterModel (segment_reduce).

Strategy (8 cores, S-column sharding for the heavy phases, pair sharding for the tail):
  - Core k owns sequence-column slice s_k = [256k, 256k+256).
  - Phase 1: entity aggregation.
      ent_att (local s-slice) = OH_mean-matmul of host-pregathered mention
      attention rows (fp8), streamed tile-by-tile, entity-half (et) outer so
      PSUM fits and the first matmul fires ~2us in.
      ent_emb^T = Ln of (exp(seq[mention_idx]) x OH_sum) matmuls emitted
      directly in [h-part, E] layout (64 N=256 matmuls); they fill the PE
      idle time under the even pair-gather window.
  - Phase 2: pair products. For all 2048 pairs: gather ent_att rows of head/
      tail entity (4KB fp8 rows cast to bf16, indirect DMA), multiply on DVE
      (bf16 2x mode), one DVE add folds 16 heads -> 8, then PE transpose-
      ACCUMULATE matmuls (x identity) fold the remaining 8 head-blocks while
      transposing -> raw^T in PSUM.
      EW = ent_emb @ W_head/W_tail is emitted between the even and odd tile
      groups so it executes under the odd gather window.
  - Phase 3: TWO AllToAlls (even pair-tiles = first 128 pairs of each dest
      core, then odd) redistribute raw^T so core k holds raw^T[:, P_k].
  - Phases 4-6 (per pair-half): contexts via seq^T-matmul, normalize, z_s/z_o
      via EW-gather one-hot matmuls + W_ctx matmuls + tanh, bilinear via
      W_bil matmuls + elementwise + ones-reduction matmul.
Host pre-casts: attention fp8 e4m3 (quantization error largely cancels in the
pair_att normalization), seq/weights bf16; mention rows (attention + seq) are
host-pregathered (pure indexing, like the one-hot/offset tensors).
DMA queues: gpsimd = pair gathers + collectives; sync = phase-1 loads,
staging, deferred tail loads (seqx/w_ctx/w_bil ride behind the even-tile
stagings so they don't steal HBM from the gathers); scalar = seqm + EW
weights; vector = paT reads (so a2a completion doesn't block other queues).
PSUM->SBUF copies ride the scalar engine to keep DVE free.
"""
import sys
import types
import numpy as np

S, H, HEADS = 2048, 1024, 16
E, NM, P = 256, 1024, 2048
PH = 1024
NC = 8
SL = S // NC          # 256 s-columns per core
PL = P // NC          # 256 pairs per core
NMT = NM // 128       # 8 mention tiles
NPT = P // 128        # 16 pair tiles
HS = HEADS * SL       # 4096 = width of per-core ent_att rows

_CACHE = {}


def _ensure_axon_profile_hook():
    """bass_utils' trace path imports antenv.axon_hooks, absent in this image."""
    if 'antenv.axon_hooks' in sys.modules:
        return
    try:
        import antenv.axon_hooks  # noqa: F401
        return
    except ImportError:
        pass
    mod = types.ModuleType('antenv.axon_hooks')
    holder = [None]
    mod.set_axon_ntff_profile_hook = lambda h: holder.__setitem__(0, h)
    mod.get_axon_ntff_profile_hook = lambda: holder[0]
    sys.modules['antenv.axon_hooks'] = mod
    try:
        from trn_agent_boot.trn_boot import _ntff_profile_via_ctypes
        hook = _ntff_profile_via_ctypes('/opt/axon/libaxon_pjrt.so')
        if hook is not None:
            mod.set_axon_ntff_profile_hook(hook)
    except Exception:
        pass


def _build(mt_ets, debug=False):
    """mt_ets: per mention-tile, tuple of entity-128-halves it touches."""
    import concourse.bass as bass
    import concourse.bacc as bacc
    import concourse.tile as tile
    from concourse import mybir
    from concourse.masks import make_identity

    F32 = mybir.dt.float32
    BF16 = mybir.dt.bfloat16
    F8 = mybir.dt.float8e4
    I32 = mybir.dt.int32
    AF = mybir.ActivationFunctionType
    OP = mybir.AluOpType

    nc = bacc.Bacc(num_devices=NC)

    # ---------------- inputs ----------------
    ag_k = nc.declare_dram_parameter("ag_k", [128, NMT * HS], F8, isOutput=False)
    seqm = nc.declare_dram_parameter("seqm", [128, NMT * H], BF16, isOutput=False)
    seqp = nc.declare_dram_parameter("seqp", [128, (S // 128) * H], BF16, isOutput=False)
    p_off = nc.declare_dram_parameter("p_off", [128, 2 * NPT], I32, isOutput=False)
    ohs = nc.declare_dram_parameter("ohs", [128, NMT * E], F8, isOutput=False)
    ohm = nc.declare_dram_parameter("ohm", [128, NMT * E], F8, isOutput=False)
    has0r = nc.declare_dram_parameter("has0r", [1, E], F32, isOutput=False)
    ohh_k = nc.declare_dram_parameter("ohh_k", [128, 2 * PL], BF16, isOutput=False)
    oht_k = nc.declare_dram_parameter("oht_k", [128, 2 * PL], BF16, isOutput=False)
    w_head = nc.declare_dram_parameter("w_head", [128, (H // 128) * PH], BF16, isOutput=False)
    w_tail = nc.declare_dram_parameter("w_tail", [128, (H // 128) * PH], BF16, isOutput=False)
    w_ctx = nc.declare_dram_parameter("w_ctx", [128, (H // 128) * PH], BF16, isOutput=False)
    w_bil = nc.declare_dram_parameter("w_bil", [128, (PH // 128) * PH], BF16, isOutput=False)
    b_head = nc.declare_dram_parameter("b_head", [128, PH // 128], F32, isOutput=False)
    b_tail = nc.declare_dram_parameter("b_tail", [128, PH // 128], F32, isOutput=False)
    b_bil = nc.declare_dram_parameter("b_bil", [1, 1], F32, isOutput=False)
    out = nc.declare_dram_parameter("out", [1, PL], F32, isOutput=True)

    dbg = {}
    if debug:
        dbg["ent_embT"] = nc.declare_dram_parameter("d_ent_embT", [H, E], BF16, isOutput=True)
        dbg["entA"] = nc.declare_dram_parameter("d_entA", [E, HS], BF16, isOutput=True)
        dbg["rawT"] = nc.declare_dram_parameter("d_rawT", [128, 2 * NPT * 128], BF16, isOutput=True)
        dbg["ctxnT"] = nc.declare_dram_parameter("d_ctxnT", [H, PL], BF16, isOutput=True)
        dbg["zrec"] = nc.declare_dram_parameter("d_zrec", [128, 2], F32, isOutput=True)
        dbg["zsT"] = nc.declare_dram_parameter("d_zsT", [PH, PL], BF16, isOutput=True)

    # internal DRAM
    entA_dram = nc.dram_tensor("entA_dram", [E, HS], F8)
    a2a_in = [nc.dram_tensor(f"a2a{h}_in", [NC, SL, 128], BF16) for h in range(2)]
    a2a_out = [nc.dram_tensor(f"a2a{h}_out", [NC, SL, 128], BF16) for h in range(2)]

    et_mts = {0: [mt for mt in range(NMT) if 0 in mt_ets[mt]],
              1: [mt for mt in range(NMT) if 1 in mt_ets[mt]]}

    with tile.TileContext(nc) as tc:
        with tc.tile_pool(name="singles", bufs=1) as singles, \
             tc.tile_pool(name="wpool", bufs=1) as wpool:
            # ---------------- phase 0: small loads (sync queue) ----------------
            p_off_t = singles.tile([128, 2 * NPT], I32)
            nc.sync.dma_start(out=p_off_t, in_=p_off[:, :])
            ohh_t = singles.tile([128, 2, PL], BF16)
            nc.sync.dma_start(out=ohh_t, in_=ohh_k[:, :])
            oht_t = singles.tile([128, 2, PL], BF16)
            nc.sync.dma_start(out=oht_t, in_=oht_k[:, :])
            bhs_t = singles.tile([128, PH // 128], F32)
            nc.sync.dma_start(out=bhs_t, in_=b_head[:, :])
            bts_t = singles.tile([128, PH // 128], F32)
            nc.sync.dma_start(out=bts_t, in_=b_tail[:, :])
            bbil_t = singles.tile([1, 1], F32)
            nc.sync.dma_start(out=bbil_t, in_=b_bil[:, :])
            ident = singles.tile([128, 128], BF16)
            make_identity(nc, ident[:, :])
            # warm activation tables; Exp last = first real user
            warm = singles.tile([1, 8], F32)
            nc.vector.memset(warm[:, :], 0.0)
            nc.scalar.activation(out=warm[:, :], in_=warm[:, :], func=AF.Tanh)
            nc.scalar.activation(out=warm[:, :], in_=warm[:, :], func=AF.Ln)
            nc.scalar.activation(out=warm[:, :], in_=warm[:, :], func=AF.Exp)
            ones_col = singles.tile([128, 1], BF16)
            nc.vector.memset(ones_col[:, :], 1.0)

            entTe = singles.tile([128, H // 128, E], BF16)  # ent_emb^T [hcol-part, hc, e]
            rawT = singles.tile([128, 2, NPT, 128], BF16)   # [s-part, sh, pt, p-row]
            paT = singles.tile([128, S // 128, PL], BF16)   # raw^T for my pairs, all s
            ctxT = singles.tile([128, H // 128, 128], BF16)
            ctxp_sb = singles.tile([128, H], BF16)          # normalized contexts [p, h]
            zsT = singles.tile([128, PH // 128, 128], BF16)
            zoT = singles.tile([128, PH // 128, 128], BF16)
            EWh = singles.tile([128, 2, PH], BF16)          # ent_emb @ W_head [e-part, et, PH]
            EWt = singles.tile([128, 2, PH], BF16)
            lg_sb = singles.tile([1, PL], F32)
            dbg_zs = singles.tile([128, PH // 128, PL], BF16) if debug else None
            dbg_ctx = singles.tile([128, H // 128, PL], BF16) if debug else None
            dbg_zr = singles.tile([128, 2], F32) if debug else None

            whb = wpool.tile([128, H // 128, PH], BF16)
            wtb = wpool.tile([128, H // 128, PH], BF16)
            seqx = wpool.tile([128, S // 128, H], BF16)
            wcb = wpool.tile([128, H // 128, PH], BF16)
            wbb = wpool.tile([128, PH // 128, PH], BF16)

            # ---------------- phase 1: aggregation + lse ----------
            with tc.tile_pool(name="p1", bufs=1) as p1:
                # seqm first on the scalar HWDGE queue (exp needs it ~5us in),
                # then the weight/seq loads (they fit under the agg window).
                entA_sb = p1.tile([128, 2, HS], F8)
                ohm_t = p1.tile([128, NMT, E], F8)
                nc.sync.dma_start(out=ohm_t, in_=ohm[:, :])
                ohs_t = p1.tile([128, NMT, E], F8)
                nc.sync.dma_start(out=ohs_t, in_=ohs[:, :])
                has0b = p1.tile([128, E], F32)
                nc.sync.dma_start(out=has0b, in_=has0r[:, :].to_broadcast([128, E]))

                # entity attention aggregation: et outer (PSUM = 2 hg x 8KB),
                # all mention tiles resident, loaded in two chunks (et0's
                # prefix first) so the et0 chains start early and the matmuls
                # run back-to-back at full PE clock.
                ag_all = p1.tile([128, NMT, HS], F8)
                split = (max(et_mts[0]) + 1) if et_mts[0] else 0
                if split > 0:
                    nc.scalar.dma_start(out=ag_all[:, 0:split, :],
                                        in_=ag_k[:, 0:split * HS])
                if split < NMT:
                    nc.scalar.dma_start(out=ag_all[:, split:NMT, :],
                                        in_=ag_k[:, split * HS:NMT * HS])
                # remaining loads fill the otherwise-idle DMA window under agg
                seqm_t = p1.tile([128, NMT, H], BF16)
                nc.scalar.dma_start(out=seqm_t, in_=seqm[:, :])
                nc.scalar.dma_start(out=whb, in_=w_head[:, :])
                nc.scalar.dma_start(out=wtb, in_=w_tail[:, :])
                nc.scalar.dma_start(out=seqx, in_=seqp[:, :])
                nc.scalar.dma_start(out=wcb, in_=w_ctx[:, :])
                nc.scalar.dma_start(out=wbb, in_=w_bil[:, :])

                with tc.tile_pool(name="ps_a", bufs=1, space="PSUM") as ps_a:
                    for et in range(2):
                        mts = et_mts[et]
                        pas0 = ps_a.tile([128, 8 * SL], F32, space="PSUM", tag="agg0")
                        pas1 = ps_a.tile([128, 8 * SL], F32, space="PSUM", tag="agg1")
                        pas = {0: pas0, 1: pas1}
                        if not mts:
                            for hg in range(2):
                                nc.vector.memset(pas[hg][:, :], 0.0)
                        for mi, mt in enumerate(mts):
                            for hg in range(2):
                                for nch in range(4):
                                    nc.tensor.matmul(
                                        pas[hg][:, nch * 512:(nch + 1) * 512],
                                        ohm_t[:, mt, et * 128:(et + 1) * 128],
                                        ag_all[:, mt, hg * 2048 + nch * 512:
                                               hg * 2048 + (nch + 1) * 512],
                                        start=(mi == 0), stop=(mi == len(mts) - 1))
                        for hg in range(2):
                            nc.scalar.copy(
                                out=entA_sb[:, et, hg * 2048:(hg + 1) * 2048],
                                in_=pas[hg][:, :])
                            nc.sync.dma_start(
                                out=entA_dram.rearrange("(t p) w -> p t w", p=128)[
                                    :, et, hg * 2048:(hg + 1) * 2048],
                                in_=entA_sb[:, et, hg * 2048:(hg + 1) * 2048])
                            if debug:
                                eAb = p1.tile([128, 8 * SL], BF16, tag="entA_dbg")
                                nc.vector.tensor_copy(out=eAb[:, :], in_=pas[hg][:, :])
                                nc.sync.dma_start(
                                    out=dbg["entA"].rearrange("(t p) w -> p t w", p=128)[
                                        :, et, hg * 2048:(hg + 1) * 2048],
                                    in_=eAb[:, :])

                # exp(seq[mention_idx]) in place on scalar (after the entA
                # copies in scalar-engine order, so they don't block agg et1)
                for mt in range(NMT):
                    nc.scalar.activation(out=seqm_t[:, mt, :], in_=seqm_t[:, mt, :],
                                         func=AF.Exp)

                # logsumexp sums, transposed layout: sums^T[h, e] =
                # sum_m exp(seq_m)[m, h] ohs[m, e]. Emitted after agg so these
                # matmuls fill PE idle time under the even pair gathers.
                with tc.tile_pool(name="ps_l", bufs=1, space="PSUM") as ps_l:
                    for hc in range(H // 128):
                        sps = ps_l.tile([128, E], F32, space="PSUM", tag=f"s{hc}")
                        for mt in range(NMT):
                            nc.tensor.matmul(
                                sps[:, :], seqm_t[:, mt, hc * 128:(hc + 1) * 128],
                                ohs_t[:, mt, :], start=(mt == 0),
                                stop=(mt == NMT - 1))
                        nc.vector.tensor_tensor(out=sps[:, :], in0=sps[:, :],
                                                in1=has0b[:, :], op=OP.add)
                        nc.scalar.activation(out=entTe[:, hc, :], in_=sps[:, :],
                                             func=AF.Ln)
                if debug:
                    nc.sync.dma_start(
                        out=dbg["ent_embT"].rearrange("(t p) e -> p t e", p=128), in_=entTe)

            # ---------------- phase 2: pair products ----------------
            # evens (tiles 0,2,..,14 = first 128 pairs of each dest core) first
            # so AllToAll #A can fire while the odds still stream.
            def pair_tile(pt, pg, prod, ps_r):
                th = pg.tile([128, HS], BF16, tag="th")
                nc.gpsimd.indirect_dma_start(
                    out=th[:, :], out_offset=None, in_=entA_dram[:, :],
                    in_offset=bass.IndirectOffsetOnAxis(
                        ap=p_off_t[:, 2 * pt:2 * pt + 1], axis=0))
                tt = pg.tile([128, HS], BF16, tag="tt")
                nc.gpsimd.indirect_dma_start(
                    out=tt[:, :], out_offset=None, in_=entA_dram[:, :],
                    in_offset=bass.IndirectOffsetOnAxis(
                        ap=p_off_t[:, 2 * pt + 1:2 * pt + 2], axis=0))
                pr = prod.tile([128, HS], BF16, tag="pr")
                nc.vector.tensor_tensor(out=pr[:, :], in0=th[:, :], in1=tt[:, :],
                                        op=OP.mult)
                # fold 16 heads -> 8 on DVE; remaining 8 fold inside the
                # transpose-accumulate matmuls (x identity) on PE.
                nc.vector.tensor_tensor(out=pr[:, :8 * SL], in0=pr[:, :8 * SL],
                                        in1=pr[:, 8 * SL:], op=OP.add)
                rp = ps_r.tile([128, 2, 128], F32, space="PSUM", tag="rp")
                for sh in range(2):
                    for hb in range(8):
                        nc.tensor.matmul(
                            rp[:, sh, :],
                            pr[:, hb * SL + sh * 128: hb * SL + sh * 128 + 128],
                            ident[:, :], start=(hb == 0), stop=(hb == 7))
                    nc.scalar.copy(out=rawT[:, sh, pt, :], in_=rp[:, sh, :])
                c, odd = pt // 2, pt % 2
                nc.sync.dma_start(
                    out=a2a_in[odd][c].rearrange("(sh sp) p -> sp sh p", sh=2),
                    in_=rawT[:, :, pt, :])

            def ew_chain(ps_e, wsb, dstw, et):
                # one (W, et) chain of EW = ent_emb @ W: ~6us of PE, spread
                # between pair tiles so no single block dams the pipeline.
                # Shares the psA "ucp" banks (sequential uses, copy-drained).
                ep = ps_e.tile([128, PH], F32, space="PSUM", tag="ucp")
                for kt in range(H // 128):
                    for nch in range(2):
                        nc.tensor.matmul(
                            ep[:, nch * 512:(nch + 1) * 512],
                            entTe[:, kt, et * 128:(et + 1) * 128],
                            wsb[:, kt, nch * 512:(nch + 1) * 512],
                            start=(kt == 0), stop=(kt == H // 128 - 1))
                nc.scalar.copy(out=dstw[:, et, :], in_=ep[:, :])

            with tc.tile_pool(name="pg", bufs=3) as pg, \
                 tc.tile_pool(name="prod", bufs=2) as prod, \
                 tc.tile_pool(name="ps_r", bufs=2, space="PSUM") as ps_r, \
                 tc.tile_pool(name="psA", bufs=1, space="PSUM") as psA, \
                 tc.tile_pool(name="psB", bufs=2, space="PSUM") as psB, \
                 tc.tile_pool(name="zscr", bufs=2) as zscr:

                # ---------------- tail pieces (per pair-half) ----------------
                def tail_z(hf):
                    q0, q1 = hf * 128, hf * 128 + 128
                    zp2 = psA.tile([128, 1], F32, space="PSUM", tag="z2")
                    for t in range(S // 128):
                        nc.tensor.matmul(
                            zp2[:, :], paT[:, t, q0:q1], ones_col[:, :],
                            start=(t == 0), stop=(t == S // 128 - 1))
                    zr = zscr.tile([128, 1], F32, tag="zr")
                    nc.vector.tensor_scalar_add(out=zr[:, :], in0=zp2[:, :],
                                                scalar1=1e-6)
                    nc.vector.reciprocal(out=zr[:, :], in_=zr[:, :])
                    if debug:
                        nc.vector.tensor_copy(out=dbg_zr[:, hf:hf + 1], in_=zr[:, :])
                    return zr

                def tail_ucp_ctx(hf, zr):
                    q0, q1 = hf * 128, hf * 128 + 128
                    ucp = psA.tile([128, H], F32, space="PSUM", tag="ucp")
                    for t in range(S // 128):
                        for nchu in range(2):
                            nc.tensor.matmul(
                                ucp[:, nchu * 512:(nchu + 1) * 512],
                                paT[:, t, q0:q1],
                                seqx[:, t, nchu * 512:(nchu + 1) * 512],
                                start=(t == 0), stop=(t == S // 128 - 1))
                    # normalize on scalar (per-partition scale), transpose back
                    nc.scalar.activation(out=ctxp_sb[:, :], in_=ucp[:, :],
                                         func=AF.Copy, scale=zr[:, :])
                    for mc in range(H // 128):
                        tw = psB.tile([128, 128], F32, space="PSUM", tag="work")
                        nc.tensor.matmul(tw[:, :], ctxp_sb[:, mc * 128:(mc + 1) * 128],
                                         ident[:, :], start=True, stop=True)
                        nc.scalar.copy(out=ctxT[:, mc, :], in_=tw[:, :])
                        if debug:
                            nc.vector.tensor_copy(out=dbg_ctx[:, mc, q0:q1],
                                                  in_=ctxT[:, mc, :])

                def tail_zs(hf, jts):
                    q0, q1 = hf * 128, hf * 128 + 128
                    for jt in jts:
                        cps = psB.tile([128, 128], F32, space="PSUM", tag="work")
                        for kt in range(H // 128):
                            nc.tensor.matmul(
                                cps[:, :], wcb[:, kt, jt * 128:(jt + 1) * 128],
                                ctxT[:, kt, :], start=(kt == 0),
                                stop=(kt == H // 128 - 1))
                        cpsb = zscr.tile([128, 128], BF16, tag="cpsb")
                        nc.scalar.copy(out=cpsb[:, :], in_=cps[:, :])
                        for (ew, oh, bias, dstz) in ((EWh, ohh_t, bhs_t, zsT),
                                                     (EWt, oht_t, bts_t, zoT)):
                            zps = psB.tile([128, 128], F32, space="PSUM", tag="work")
                            for et in range(2):
                                nc.tensor.matmul(
                                    zps[:, :], ew[:, et, jt * 128:(jt + 1) * 128],
                                    oh[:, et, q0:q1], start=(et == 0), stop=(et == 1))
                            nc.vector.tensor_tensor(out=zps[:, :], in0=zps[:, :],
                                                    in1=cpsb[:, :], op=OP.add)
                            nc.scalar.activation(out=dstz[:, jt, :], in_=zps[:, :],
                                                 func=AF.Tanh, bias=bias[:, jt:jt + 1])
                        if debug:
                            nc.vector.tensor_copy(out=dbg_zs[:, jt, q0:q1],
                                                  in_=zsT[:, jt, :])

                def tail_bil(hf):
                    q0, q1 = hf * 128, hf * 128 + 128
                    lg = psA.tile([1, 128], F32, space="PSUM", tag="lg")
                    for jt in range(PH // 128):
                        ups = psB.tile([128, 128], F32, space="PSUM", tag="work")
                        for it in range(PH // 128):
                            nc.tensor.matmul(
                                ups[:, :], wbb[:, it, jt * 128:(jt + 1) * 128],
                                zsT[:, it, :], start=(it == 0),
                                stop=(it == PH // 128 - 1))
                        pb = zscr.tile([128, 128], BF16, tag="pb")
                        nc.vector.tensor_tensor(out=pb[:, :], in0=ups[:, :],
                                                in1=zoT[:, jt, :], op=OP.mult)
                        nc.tensor.matmul(
                            lg[:, :], ones_col[:, :], pb[:, :],
                            start=(jt == 0), stop=(jt == PH // 128 - 1))
                    nc.vector.tensor_scalar_add(out=lg_sb[:, q0:q1], in0=lg[:, :],
                                                scalar1=bbil_t[:, 0:1])

                # ---------------- even pair tiles + EW chains ----------------
                for c in range(NC):
                    pair_tile(2 * c, pg, prod, ps_r)
                    if c == 2:
                        ew_chain(psA, whb, EWh, 0)
                    elif c == 4:
                        ew_chain(psA, whb, EWh, 1)
                    elif c == 6:
                        ew_chain(psA, wtb, EWt, 0)
                # a2a #A fires as soon as the evens are staged.
                nc.gpsimd.collective_compute(
                    "AllToAll", OP.bypass, replica_groups=[list(range(NC))],
                    ins=[a2a_in[0][:, :, :]], outs=[a2a_out[0][:, :, :]])
                nc.scalar.dma_start(
                    out=paT[:, :, 0:128],
                    in_=a2a_out[0].rearrange("j (sh sp) q -> sp (j sh) q", sh=2))

                # ---------------- odd pair tiles + EWt + tail half 0 ---------
                zr0 = None
                for c in range(NC):
                    pair_tile(2 * c + 1, pg, prod, ps_r)
                    if c == 0:
                        ew_chain(psA, wtb, EWt, 1)
                    elif c == 4:
                        zr0 = tail_z(0)
                    elif c == 5:
                        tail_ucp_ctx(0, zr0)
                    elif c == 6:
                        tail_zs(0, range(0, 4))
                    elif c == 7:
                        tail_zs(0, range(4, 8))
                nc.gpsimd.collective_compute(
                    "AllToAll", OP.bypass, replica_groups=[list(range(NC))],
                    ins=[a2a_in[1][:, :, :]], outs=[a2a_out[1][:, :, :]])
                nc.scalar.dma_start(
                    out=paT[:, :, 128:256],
                    in_=a2a_out[1].rearrange("j (sh sp) q -> sp (j sh) q", sh=2))
                tail_bil(0)
                if debug:
                    nc.sync.dma_start(
                        out=dbg["rawT"][:, :],
                        in_=rawT.rearrange("p a b c -> p (a b c)"))

                # ---------------- tail half 1 ----------------
                zr1 = tail_z(1)
                tail_ucp_ctx(1, zr1)
                tail_zs(1, range(PH // 128))
                tail_bil(1)
                if debug:
                    nc.sync.dma_start(
                        out=dbg["ctxnT"].rearrange("(t p) q -> p t q", p=128),
                        in_=dbg_ctx)
                    nc.sync.dma_start(out=dbg["zrec"][:, :], in_=dbg_zr)
                    nc.sync.dma_start(
                        out=dbg["zsT"].rearrange("(t p) q -> p t q", p=128), in_=dbg_zs)
                nc.sync.dma_start(out=out[:, :], in_=lg_sb)

    nc.finalize()
    return nc


def _get_nc(mt_ets, debug=False):
    key = ("nc", mt_ets, debug)
    if key not in _CACHE:
        _CACHE[key] = _build(mt_ets, debug)
    return _CACHE[key]


def _prep_in_maps(inputs):
    import ml_dtypes
    bf16 = ml_dtypes.bfloat16
    f8 = ml_dtypes.float8_e4m3

    att = np.asarray(inputs["attention"], np.float32)          # [16, 2048, 2048]
    seq = np.asarray(inputs["sequence_output"], np.float32)
    mention_idx = np.asarray(inputs["mention_idx"], np.int32)  # [1024]
    entity_ids = np.asarray(inputs["entity_ids"], np.int32)    # [1024]
    pair_h = np.asarray(inputs["pair_h"], np.int32)            # [2048]
    pair_t = np.asarray(inputs["pair_t"], np.int32)

    def pm(x, t):
        """[(t*128), f...] -> partition-major [128, t*f] contiguous rows."""
        f = x.size // (t * 128)
        return np.ascontiguousarray(
            x.reshape(t, 128, f).transpose(1, 0, 2)).reshape(128, t * f)

    counts = np.bincount(entity_ids, minlength=E).astype(np.float32)
    inv_cnt = 1.0 / np.maximum(counts, 1.0)

    ohm = np.zeros((NM, E), np.float32)
    ohm[np.arange(NM), entity_ids] = inv_cnt[entity_ids]
    ohs_np = np.zeros((NM, E), np.float32)
    ohs_np[np.arange(NM), entity_ids] = 1.0
    has0r = (counts == 0).astype(np.float32)[None, :]

    # which entity-128-halves each mention tile touches (all-zero slabs skipped)
    mt_ets = tuple(
        tuple(sorted(set((entity_ids[mt * 128:(mt + 1) * 128] // 128).tolist())))
        for mt in range(NMT))

    order = np.argsort(pair_h, kind="stable")
    sph = pair_h[order]
    spt = pair_t[order]
    p_off = np.zeros((128, 2 * NPT), np.int32)
    for pt in range(NPT):
        seg = slice(pt * 128, (pt + 1) * 128)
        p_off[:, 2 * pt] = sph[seg]
        p_off[:, 2 * pt + 1] = spt[seg]

    # host-pregathered mention rows (pure indexing + dtype cast)
    att8_m = att[:, mention_idx, :].astype(f8)                 # [16, NM, 2048]
    seq_m = seq[mention_idx].astype(bf16)                      # [NM, H]

    shared = {
        "seqm": pm(seq_m, NMT),
        "seqp": pm(seq.astype(bf16), S // 128),
        "p_off": p_off,
        "ohm": pm(ohm.astype(f8), NMT),
        "ohs": pm(ohs_np.astype(f8), NMT),
        "has0r": has0r,
        "w_head": pm(np.asarray(inputs["W_head"], np.float32).astype(bf16), H // 128),
        "w_tail": pm(np.asarray(inputs["W_tail"], np.float32).astype(bf16), H // 128),
        "w_ctx": pm(np.asarray(inputs["W_ctx"], np.float32).astype(bf16), H // 128),
        "w_bil": pm(np.asarray(inputs["W_bil"], np.float32).astype(bf16), PH // 128),
        "b_head": np.asarray(inputs["b_head"], np.float32).reshape(PH // 128, 128).T.copy(),
        "b_tail": np.asarray(inputs["b_tail"], np.float32).reshape(PH // 128, 128).T.copy(),
        "b_bil": np.asarray(inputs["b_bil"], np.float32).reshape(1, 1),
    }

    in_maps = []
    for k in range(NC):
        sk = k * SL
        ag_kk = np.ascontiguousarray(
            att8_m[:, :, sk:sk + SL].transpose(1, 0, 2)).reshape(NM, HS)
        ohh_kk = np.zeros((E, PL), np.float32)
        ohh_kk[sph[k * PL:(k + 1) * PL], np.arange(PL)] = 1.0
        oht_kk = np.zeros((E, PL), np.float32)
        oht_kk[spt[k * PL:(k + 1) * PL], np.arange(PL)] = 1.0
        m = dict(shared)
        m["ag_k"] = pm(ag_kk, NMT)
        m["ohh_k"] = pm(ohh_kk.astype(bf16), 2)
        m["oht_k"] = pm(oht_kk.astype(bf16), 2)
        in_maps.append(m)
    return in_maps, mt_ets


def _run(inputs, trace=False, debug=False):
    _ensure_axon_profile_hook()
    from concourse.bass_utils import run_bass_kernel_spmd
    in_maps, mt_ets = _prep_in_maps(inputs)
    nc = _get_nc(mt_ets, debug)
    res = run_bass_kernel_spmd(nc, in_maps, list(range(NC)), trace=trace)
    sorted_logits = np.concatenate([np.asarray(res.results[k]["out"][0], np.float32)
                                    for k in range(NC)])
    order = np.argsort(np.asarray(inputs["pair_h"], np.int32), kind="stable")
    logits = np.empty(P, np.float32)
    logits[order] = sorted_logits
    return logits, res


def kernel(**inputs) -> np.ndarray:
    logits, _ = _run(inputs, trace=False)
    return logits


def kernel_traced(**inputs):
    logits, res = _run(inputs, trace=True)
    return logits, res


def kernel_debug(**inputs):
    logits, res = _run(inputs, trace=False, debug=True)
    return logits, res
